# revision 17
# baseline (speedup 1.0000x reference)
"""DeepSeek-style GQA attention block (B=2, S=2048, H=1536, 12 q-heads /
2 kv-heads, d=128) sharded over 8 TRN2 NeuronCores.

Sharding: core = (batch b, kv-group hh, query-half th).
  - tensor parallel over the 2 kv groups (6 q-heads + 1 kv head each)
  - data parallel over batch (2)
  - query-token parallel (2 halves of 1024)
Each core computes its 6 heads' attention for its 1024 query tokens against
the full 2048-token K/V of its kv head, then a partial O-projection; the two
kv-group partials per (b, th) are summed on the host.

All matmuls run in fp16 with fp32 PSUM accumulation (fp16 has the same PE
throughput as bf16 but 4 more mantissa bits; every tensor here is O(1) so
fp16 range is ample). Softmax runs without max-subtraction, with the
1/sqrt(d) scale and the additive mask bias fused into the ACT exp.

Layout trick: scores are computed TRANSPOSED (scores^T[Sk, Sq] = K Q^T) so
the attention probabilities come out with Sk on partitions — exactly the
layout the AV matmul needs as its moving operand.

The softmax denominator: exp chunks are folded on the DVE (fp16 adds);
only two tiny ones-matmuls per head (partition sum + broadcast) touch the
tensor engine.  The bulk rowsum that a naive transposed-softmax needs
(one extra full pass of e through the PE) is gone entirely.

Pipelining: as soon as a head's AV accumulation stops, one DVE copy moves
the raw result to SBUF, freeing the single AV PSUM buffer early; the
normalization (reciprocal+multiply) finishes in the background.  Each
head's last AV chunk and rowsum are deferred into the next head's window,
and the next head's Q projection matmuls are spread through the chunk
loop as PE filler, because the exp stream (1.11us/chunk on the scalar
engine) is slower than the per-chunk score+AV matmuls (0.86us).
"""

import numpy as np
import ml_dtypes

HIDDEN = 1536
D = 128          # head dim
NH = 6           # q-heads per core
B, S = 2, 2048
SQ = 1024        # query tokens per core
HC = HIDDEN // 128   # 12 hidden chunks
SKC = S // 128       # 16 key chunks
SCALE = float(1.0 / np.sqrt(np.float32(D)))

_NC_CACHE = {}
last_results = None  # BassKernelResults of the most recent run (for test.py)


def _build_nc():
    import concourse.bacc as bacc
    import concourse.mybir as mybir
    import concourse.tile as tile
    from concourse.masks import make_identity

    f16 = mybir.dt.float16
    f32 = mybir.dt.float32
    Exp = mybir.ActivationFunctionType.Exp
    Add = mybir.AluOpType.add

    nc = bacc.Bacc("TRN2", target_bir_lowering=False, debug=False, num_devices=8)

    xt = nc.dram_tensor("xt", [HIDDEN, S], f16, kind="ExternalInput")
    wqt = nc.dram_tensor("wqt", [HIDDEN, NH * D], f16, kind="ExternalInput")
    wkt = nc.dram_tensor("wkt", [HIDDEN, D], f16, kind="ExternalInput")
    wvt = nc.dram_tensor("wvt", [HIDDEN, D], f16, kind="ExternalInput")
    wot = nc.dram_tensor("wot", [NH * D, HIDDEN], f16, kind="ExternalInput")
    biasd = nc.dram_tensor("biasd", [128, SKC], f32, kind="ExternalInput")
    y = nc.dram_tensor("y", [SQ, HIDDEN], f16, kind="ExternalOutput")

    with tile.TileContext(nc) as tc:
        with (
            tc.tile_pool(name="const", bufs=1) as constp,
            tc.tile_pool(name="weights", bufs=1) as wp,
            tc.tile_pool(name="persist", bufs=1) as pers,
        ):
            ident = constp.tile([128, 128], f16)
            make_identity(nc, ident[:])
            ones_col = constp.tile([128, 1], f16)
            nc.vector.memset(ones_col[:], 1.0)
            ones_row = constp.tile([1, 128], f16)
            nc.vector.memset(ones_row[:], 1.0)
            bias_sb = constp.tile([128, SKC], f32)

            wq_sb = wp.tile([128, HC, NH * D], f16)
            wk_sb = wp.tile([128, HC, D], f16)
            wv_sb = wp.tile([128, HC, D], f16)
            wo_sb = wp.tile([128, NH, HIDDEN], f16)

            xt_sb = pers.tile([128, HC, S], f16)
            kT_sb = pers.tile([128, S], f16)          # K^T [d, Sk]
            v_sb = pers.tile([128, SKC, D], f16)      # V [Sk, d], chunked
            qT_sb = pers.tile([128, NH, SQ], f16)     # Q^T [d, Sq] per head
            outT_sb = pers.tile([128, NH, SQ], f16)   # AV out^T [d, Sq] per head
            e_sb = pers.tile([128, SKC, SQ], f16)     # exp(scores^T) per head

            # DMA order = need order: wk/wv gate the first matmuls, then the
            # xt chunk stream paces the K/V projection, then wq (Q proj),
            # then wo (O proj, needed last).  bias is tiny, needed at the
            # first exp.
            nc.sync.dma_start(wk_sb[:], wkt.ap().rearrange("(c p) m -> p c m", p=128))
            # first chunk split in half so the first K matmul starts sooner
            nc.sync.dma_start(xt_sb[:, 0, 0:1024], xt[0:128, 0:1024])
            nc.sync.dma_start(xt_sb[:, 0, 1024:2048], xt[0:128, 1024:2048])
            nc.sync.dma_start(wv_sb[:], wvt.ap().rearrange("(c p) m -> p c m", p=128))
            nc.sync.dma_start(xt_sb[:, 1, :], xt[128:256, :])
            nc.sync.dma_start(bias_sb[:], biasd.ap())
            for c in range(2, HC):
                nc.sync.dma_start(xt_sb[:, c, :], xt[128 * c : 128 * (c + 1), :])
            wqt_r = wqt.ap().rearrange("(c p) m -> p c m", p=128)
            for c in range(HC):
                nc.sync.dma_start(wq_sb[:, c, :], wqt_r[:, c, :])
            nc.sync.dma_start(wo_sb[:], wot.ap().rearrange("(h p) n -> p h n", p=128))

            # ---------- phase 1: K/V/Q0 projections ----------
            # K uses 4 PSUM banks, V only 2 (two Sk-half passes) so head 0's
            # Q projection can accumulate in the remaining 2 banks DURING the
            # DMA-paced xt chunk stream: every chunk arrival feeds 10
            # matmuls, and the attention phase starts ~9us earlier.
            with tc.tile_pool(name="vtp", bufs=1) as vtp:
                vT_sb = vtp.tile([128, S], f16)
                with (
                    tc.tile_pool(name="kv_ps", bufs=1, space="PSUM") as kvps,
                    tc.tile_pool(name="q0_ps", bufs=1, space="PSUM") as q0ps,
                ):
                    kps = kvps.tile([128, 4, 512], f32, tag="kps")
                    vps = kvps.tile([128, 2, 512], f32, tag="vps")
                    q0 = q0ps.tile([128, SQ], f32, tag="q0")
                    for c in range(HC):
                        for sb in range(4):
                            nc.tensor.matmul(
                                kps[:, sb, :],
                                wk_sb[:, c, :],
                                xt_sb[:, c, 512 * sb : 512 * (sb + 1)],
                                start=(c == 0),
                                stop=(c == HC - 1),
                            )
                        for sb in range(2):
                            nc.tensor.matmul(
                                vps[:, sb, :],
                                wv_sb[:, c, :],
                                xt_sb[:, c, 512 * sb : 512 * (sb + 1)],
                                start=(c == 0),
                                stop=(c == HC - 1),
                            )
                        for sqh in range(2):
                            nc.tensor.matmul(
                                q0[:, 512 * sqh : 512 * (sqh + 1)],
                                wq_sb[:, c, 0:D],
                                xt_sb[:, c, 512 * sqh : 512 * (sqh + 1)],
                                start=(c == 0),
                                stop=(c == HC - 1),
                            )
                    # drain order: kT block 0 + qT(0) first (they gate the
                    # first scores), then vT half 0 (gates the transposes).
                    nc.vector.tensor_copy(kT_sb[:, 0:512], kps[:, 0, :])
                    nc.vector.tensor_copy(qT_sb[:, 0, :], q0[:])
                    for sb in range(2):
                        nc.vector.tensor_copy(
                            vT_sb[:, 512 * sb : 512 * (sb + 1)], vps[:, sb, :]
                        )
                    for sb in range(1, 4):
                        nc.vector.tensor_copy(
                            kT_sb[:, 512 * sb : 512 * (sb + 1)], kps[:, sb, :]
                        )
                    # V second half in the freed vps banks
                    vps2 = kvps.tile([128, 2, 512], f32, tag="vps")
                    for c in range(HC):
                        for sb in range(2):
                            nc.tensor.matmul(
                                vps2[:, sb, :],
                                wv_sb[:, c, :],
                                xt_sb[:, c, 512 * (sb + 2) : 512 * (sb + 3)],
                                start=(c == 0),
                                stop=(c == HC - 1),
                            )
                    for sb in range(2):
                        nc.vector.tensor_copy(
                            vT_sb[:, 512 * (sb + 2) : 512 * (sb + 3)],
                            vps2[:, sb, :],
                        )

                # ---------- phase 2: V transpose ----------
                with tc.tile_pool(name="proj_ps", bufs=2, space="PSUM") as pps:
                    for c in range(SKC):
                        pt = pps.tile([128, 128], f16, tag="vtr")
                        nc.tensor.transpose(
                            pt[:], vT_sb[:, 128 * c : 128 * (c + 1)], ident[:]
                        )
                        nc.vector.tensor_copy(v_sb[:, c, :], pt[:])

            # ---------- phase 3: attention (Q proj pipelined per head) ----
            with (
                tc.tile_pool(name="sc_ps", bufs=2, space="PSUM") as scp,
                tc.tile_pool(name="q_ps", bufs=1, space="PSUM") as qps,
                tc.tile_pool(name="av_ps", bufs=1, space="PSUM") as avp,
                tc.tile_pool(name="accp", bufs=2) as accp,
                tc.tile_pool(name="dnp", bufs=1) as dnp,
                tc.tile_pool(name="avsbp", bufs=2) as avsbp,
            ):
                def emit_scores(h, c):
                    # scores^T chunk [Sk 128, Sq 1024] = (K^T slice)^T Q^T,
                    # then e = exp(scale * scores + mask_bias) on the scalar
                    # engine (bias is per-Sk = per-partition).
                    sc = scp.tile([128, SQ], f32, tag="sc")
                    for sqh in range(2):
                        nc.tensor.matmul(
                            sc[:, 512 * sqh : 512 * (sqh + 1)],
                            kT_sb[:, 128 * c : 128 * (c + 1)],
                            qT_sb[:, h, 512 * sqh : 512 * (sqh + 1)],
                            start=True,
                            stop=True,
                        )
                    nc.scalar.activation(
                        e_sb[:, c, :],
                        sc[:],
                        Exp,
                        bias=bias_sb[:, c : c + 1],
                        scale=SCALE,
                    )

                def emit_qproj_mm(h):
                    # Q^T for head h (queries are columns 0..SQ-1 of the
                    # rolled xt); the PSUM->SBUF copy is emitted separately
                    # so its DVE slot lands after the previous head's
                    # normalization ops.
                    ps = qps.tile([128, SQ], f32, tag="qp")
                    for sqh in range(2):
                        for c in range(HC):
                            nc.tensor.matmul(
                                ps[:, 512 * sqh : 512 * (sqh + 1)],
                                wq_sb[:, c, D * h : D * (h + 1)],
                                xt_sb[:, c, 512 * sqh : 512 * (sqh + 1)],
                                start=(c == 0),
                                stop=(c == HC - 1),
                            )
                    return ps

                def emit_qproj_copy(h, ps):
                    nc.vector.tensor_copy(qT_sb[:, h, :], ps[:])

                # Normalization structure per head h (emitted in head h+1's
                # window so everything overlaps the matmul stream):
                #   rawcopy:   av PSUM -> SBUF (frees the single AV buffer)
                #   rs-mm:     ones-col matmul partition-sums the exp acc
                #              (emitted at the end of window h)
                #   rs copy:   [1,1024] PSUM -> SBUF fp16
                #   bc-mm:     ones-row matmul broadcasts it to 128 partitions
                #   recip+mul: DVE; recip is emitted before the qcopy so the
                #              sc-slot reuse by later score tiles never waits.
                def emit_rs(acc):
                    rs = scp.tile([128, SQ], f32, tag="sc")
                    for sqh in range(2):
                        nc.tensor.matmul(
                            rs[0:1, 512 * sqh : 512 * (sqh + 1)],
                            ones_col[:],
                            acc[:, 512 * sqh : 512 * (sqh + 1)],
                            start=True,
                            stop=True,
                        )
                    return rs

                def tail_pre(pav, pavsb, prs):
                    # window-start part: free AV PSUM, stage rowsums in SBUF
                    nc.vector.tensor_copy(pavsb[:], pav[:])
                    prs_sb = dnp.tile([1, SQ], f16, tag="rssb")
                    nc.vector.tensor_copy(prs_sb[:], prs[0:1, :])
                    return prs_sb

                def tail_bc(prs_sb):
                    bc = scp.tile([128, SQ], f32, tag="sc")
                    for sqh in range(2):
                        nc.tensor.matmul(
                            bc[:, 512 * sqh : 512 * (sqh + 1)],
                            ones_row[:],
                            prs_sb[:, 512 * sqh : 512 * (sqh + 1)],
                            start=True,
                            stop=True,
                        )
                    return bc

                def tail_norm(ph, pavsb, bc):
                    prden = dnp.tile([128, SQ], f32, tag="rden")
                    nc.vector.reciprocal_approx_fast(prden[:], bc[:])
                    nc.vector.tensor_mul(outT_sb[:, ph, :], pavsb[:], prden[:])

                def emit_qproj_part(h, ps, c0, c1):
                    for sqh in range(2):
                        for c in range(c0, c1):
                            nc.tensor.matmul(
                                ps[:, 512 * sqh : 512 * (sqh + 1)],
                                wq_sb[:, c, D * h : D * (h + 1)],
                                xt_sb[:, c, 512 * sqh : 512 * (sqh + 1)],
                                start=(c == 0),
                                stop=(c == HC - 1),
                            )

                def emit_av(av, c, stop):
                    for sqh in range(2):
                        nc.tensor.matmul(
                            av[:, 512 * sqh : 512 * (sqh + 1)],
                            v_sb[:, c, :],
                            e_sb[:, c, 512 * sqh : 512 * (sqh + 1)],
                            start=(c == 0),
                            stop=stop,
                        )

                def finish_head(pav, pacc):
                    # deferred last AV chunk (its exp is ready by now) and
                    # final acc fold, then the partition-sum matmuls
                    emit_av(pav, SKC - 1, True)
                    nc.vector.tensor_tensor(
                        pacc[0][:], e_sb[:, SKC - 1, :], pacc[1][:], Add
                    )

                prev = None
                for h in range(NH):
                    av = avp.tile([128, SQ], f32, tag="av")
                    accA = accp.tile([128, SQ], f16, tag="accA")
                    accB = accp.tile([128, SQ], f16, tag="accB")
                    acc = [accA, accB]
                    avsb = avsbp.tile([128, SQ], f16, tag="avsb")

                    emit_scores(h, 0)
                    qp_ps = (
                        qps.tile([128, SQ], f32, tag="qp", name=f"qp{h + 1}")
                        if h + 1 < NH
                        else None
                    )
                    if prev is not None:
                        # previous head's finish + normalization, interleaved
                        # with this head's Q-proj so the PE never waits on the
                        # scalar/DVE chain.
                        ph, pav, pacc, pavsb = prev
                        if qp_ps is not None:
                            emit_qproj_part(h + 1, qp_ps, 0, 2)
                        finish_head(pav, pacc)
                        if qp_ps is not None:
                            emit_qproj_part(h + 1, qp_ps, 2, 4)
                        prs = emit_rs(pacc[0])
                        nc.vector.tensor_copy(pavsb[:], pav[:])
                        prs_sb = dnp.tile([1, SQ], f16, tag="rssb")
                        nc.vector.tensor_copy(prs_sb[:], prs[0:1, :])
                        if qp_ps is not None:
                            emit_qproj_part(h + 1, qp_ps, 4, 6)
                        bc = tail_bc(prs_sb)
                        emit_scores(h, 1)
                        tail_norm(ph, pavsb, bc)
                        qp_done = 6
                    else:
                        if qp_ps is not None:
                            emit_qproj_part(h + 1, qp_ps, 0, 6)
                        emit_scores(h, 1)
                        qp_done = 6

                    # remaining Q-proj chunks are spread through the chunk
                    # loop: the exp stream (1.11us/chunk) is slower than the
                    # per-chunk sc+av matmuls (0.86us), so without filler the
                    # PE drains its runway and stalls ~100ns on every exp.
                    for c in range(SKC - 1):
                        emit_av(av, c, False)
                        if c + 2 < SKC:
                            emit_scores(h, c + 2)
                        if qp_ps is not None and c < 6:
                            emit_qproj_part(h + 1, qp_ps, qp_done + c, qp_done + c + 1)
                        if c == 6 and qp_ps is not None:
                            emit_qproj_copy(h + 1, qp_ps)
                        if c >= 1:
                            # acc[(c+1)%2] = e_c + (previous acc / e_0)
                            src1 = e_sb[:, 0, :] if c == 1 else acc[c % 2][:]
                            nc.vector.tensor_tensor(
                                acc[(c + 1) % 2][:], e_sb[:, c, :], src1, Add
                            )

                    prev = (h, av, acc, avsb)

                # epilogue: finish head 5 and normalize it
                lh, lav, lacc, lavsb = prev
                finish_head(lav, lacc)
                lrs = emit_rs(lacc[0])
                nc.vector.tensor_copy(lavsb[:], lav[:])
                lrs_sb = dnp.tile([1, SQ], f16, tag="rssb")
                nc.vector.tensor_copy(lrs_sb[:], lrs[0:1, :])
                tail_norm(lh, lavsb, tail_bc(lrs_sb))

            # ---------- phase 4: O-projection (partial; host sums groups) ----
            with (
                tc.tile_pool(name="y_ps", bufs=2, space="PSUM") as yp,
                tc.tile_pool(name="y_sb", bufs=3) as ysb,
            ):
                for t in range(SQ // 128):
                    for nb in range(HIDDEN // 512):
                        ps = yp.tile([128, 512], f32, tag="y")
                        for h in range(NH):
                            nc.tensor.matmul(
                                ps[:],
                                outT_sb[:, h, 128 * t : 128 * (t + 1)],
                                wo_sb[:, h, 512 * nb : 512 * (nb + 1)],
                                start=(h == 0),
                                stop=(h == NH - 1),
                            )
                        ysb_t = ysb.tile([128, 512], f16, tag="ysb")
                        nc.vector.tensor_copy(ysb_t[:], ps[:])
                        nc.sync.dma_start(
                            y[128 * t : 128 * (t + 1), 512 * nb : 512 * (nb + 1)],
                            ysb_t[:],
                        )

    nc.compile()
    return nc


def _get_nc():
    if "nc" not in _NC_CACHE:
        _NC_CACHE["nc"] = _build_nc()
    return _NC_CACHE["nc"]


def kernel(hidden_states, attention_mask, Wq, Wk, Wv, Wo):
    global last_results
    from concourse.bass_utils import run_bass_kernel_spmd

    f16 = np.float16
    hidden_states = np.asarray(hidden_states, dtype=np.float32)
    attention_mask = np.asarray(attention_mask, dtype=np.float32)
    Wq = np.asarray(Wq, dtype=np.float32)
    Wk = np.asarray(Wk, dtype=np.float32)
    Wv = np.asarray(Wv, dtype=np.float32)
    Wo = np.asarray(Wo, dtype=np.float32)

    nc = _get_nc()

    in_maps = []
    cores = []
    for b in range(2):
        xt_full = np.ascontiguousarray(hidden_states[b].T).astype(f16)  # [H, S]
        bias_full = ((1.0 - attention_mask[b]) * -10000.0).astype(np.float32)
        for hh in range(2):
            wqt = np.ascontiguousarray(
                Wq[NH * D * hh : NH * D * (hh + 1), :].T
            ).astype(f16)
            wkt = np.ascontiguousarray(Wk[D * hh : D * (hh + 1), :].T).astype(f16)
            wvt = np.ascontiguousarray(Wv[D * hh : D * (hh + 1), :].T).astype(f16)
            wot = np.ascontiguousarray(
                Wo[:, NH * D * hh : NH * D * (hh + 1)].T
            ).astype(f16)
            for th in range(2):
                # roll tokens so this core's queries are columns 0..SQ-1
                r = th * SQ
                xt_r = np.ascontiguousarray(
                    np.concatenate([xt_full[:, r:], xt_full[:, :r]], axis=1)
                )
                bias_r = np.concatenate([bias_full[r:], bias_full[:r]])
                biasd = np.ascontiguousarray(
                    bias_r.reshape(SKC, 128).T
                ).astype(np.float32)
                in_maps.append(
                    {
                        "xt": xt_r,
                        "wqt": wqt,
                        "wkt": wkt,
                        "wvt": wvt,
                        "wot": wot,
                        "biasd": biasd,
                    }
                )
                cores.append((b, hh, th))

    res = run_bass_kernel_spmd(nc, in_maps, core_ids=list(range(8)))
    last_results = res

    out = np.zeros((B, S, HIDDEN), dtype=np.float32)
    for (b, hh, th), r in zip(cores, res.results):
        out[b, th * SQ : (th + 1) * SQ, :] += r["y"].astype(np.float32)
    return out


# revision 18
# speedup vs baseline: 1.0082x; 1.0082x over previous
"""DeepSeek-style GQA attention block (B=2, S=2048, H=1536, 12 q-heads /
2 kv-heads, d=128) sharded over 8 TRN2 NeuronCores.

Sharding: core = (batch b, kv-group hh, query-half th).
  - tensor parallel over the 2 kv groups (6 q-heads + 1 kv head each)
  - data parallel over batch (2)
  - query-token parallel (2 halves of 1024)
Each core computes its 6 heads' attention for its 1024 query tokens against
the full 2048-token K/V of its kv head, then a partial O-projection; the two
kv-group partials per (b, th) are summed on the host.

All matmuls run in fp16 with fp32 PSUM accumulation (fp16 has the same PE
throughput as bf16 but 4 more mantissa bits; every tensor here is O(1) so
fp16 range is ample). Softmax runs without max-subtraction, with the
1/sqrt(d) scale and the additive mask bias fused into the ACT exp.

Layout trick: scores are computed TRANSPOSED (scores^T[Sk, Sq] = K Q^T) so
the attention probabilities come out with Sk on partitions — exactly the
layout the AV matmul needs as its moving operand.

The softmax denominator: exp chunks are folded on the DVE (fp16 adds);
only two tiny ones-matmuls per head (partition sum + broadcast) touch the
tensor engine.  The bulk rowsum that a naive transposed-softmax needs
(one extra full pass of e through the PE) is gone entirely.

Pipelining: as soon as a head's AV accumulation stops, one DVE copy moves
the raw result to SBUF, freeing the single AV PSUM buffer early; the
normalization (reciprocal+multiply) finishes in the background.  Each
head's last AV chunk and rowsum are deferred into the next head's window,
and the next head's Q projection matmuls are spread through the chunk
loop as PE filler, because the exp stream (1.11us/chunk on the scalar
engine) is slower than the per-chunk score+AV matmuls (0.86us).
"""

import numpy as np
import ml_dtypes

HIDDEN = 1536
D = 128          # head dim
NH = 6           # q-heads per core
B, S = 2, 2048
SQ = 1024        # query tokens per core
HC = HIDDEN // 128   # 12 hidden chunks
SKC = S // 128       # 16 key chunks
SCALE = float(1.0 / np.sqrt(np.float32(D)))

_NC_CACHE = {}
last_results = None  # BassKernelResults of the most recent run (for test.py)


def _build_nc():
    import concourse.bacc as bacc
    import concourse.mybir as mybir
    import concourse.tile as tile
    from concourse.masks import make_identity

    f16 = mybir.dt.float16
    f32 = mybir.dt.float32
    Exp = mybir.ActivationFunctionType.Exp
    Add = mybir.AluOpType.add

    nc = bacc.Bacc("TRN2", target_bir_lowering=False, debug=False, num_devices=8)

    xt = nc.dram_tensor("xt", [HIDDEN, S], f16, kind="ExternalInput")
    wqt = nc.dram_tensor("wqt", [HIDDEN, NH * D], f16, kind="ExternalInput")
    wkt = nc.dram_tensor("wkt", [HIDDEN, D], f16, kind="ExternalInput")
    wvt = nc.dram_tensor("wvt", [HIDDEN, D], f16, kind="ExternalInput")
    wot = nc.dram_tensor("wot", [NH * D, HIDDEN], f16, kind="ExternalInput")
    biasd = nc.dram_tensor("biasd", [128, SKC], f32, kind="ExternalInput")
    y = nc.dram_tensor("y", [SQ, HIDDEN], f16, kind="ExternalOutput")

    with tile.TileContext(nc) as tc:
        with (
            tc.tile_pool(name="const", bufs=1) as constp,
            tc.tile_pool(name="weights", bufs=1) as wp,
            tc.tile_pool(name="persist", bufs=1) as pers,
        ):
            ident = constp.tile([128, 128], f16)
            make_identity(nc, ident[:])
            ones_col = constp.tile([128, 1], f16)
            nc.vector.memset(ones_col[:], 1.0)
            ones_row = constp.tile([1, 128], f16)
            nc.vector.memset(ones_row[:], 1.0)
            bias_sb = constp.tile([128, SKC], f32)

            wq_sb = wp.tile([128, HC, NH * D], f16)
            wk_sb = wp.tile([128, HC, D], f16)
            wv_sb = wp.tile([128, HC, D], f16)
            wo_sb = wp.tile([128, NH, HIDDEN], f16)

            xt_sb = pers.tile([128, HC, S], f16)
            kT_sb = pers.tile([128, S], f16)          # K^T [d, Sk]
            v_sb = pers.tile([128, SKC, D], f16)      # V [Sk, d], chunked
            qT_sb = pers.tile([128, NH, SQ], f16)     # Q^T [d, Sq] per head
            outT_sb = pers.tile([128, NH, SQ], f16)   # AV out^T [d, Sq] per head
            e_sb = pers.tile([128, SKC, SQ], f16)     # exp(scores^T) per head

            # DMA order = need order: wk/wv gate the first matmuls, then the
            # xt chunk stream paces the K/V projection, then wq (Q proj),
            # then wo (O proj, needed last).  bias is tiny, needed at the
            # first exp.
            nc.sync.dma_start(wk_sb[:], wkt.ap().rearrange("(c p) m -> p c m", p=128))
            # first chunk split in half so the first K matmul starts sooner
            nc.sync.dma_start(xt_sb[:, 0, 0:1024], xt[0:128, 0:1024])
            nc.sync.dma_start(xt_sb[:, 0, 1024:2048], xt[0:128, 1024:2048])
            nc.sync.dma_start(wv_sb[:], wvt.ap().rearrange("(c p) m -> p c m", p=128))
            nc.sync.dma_start(xt_sb[:, 1, :], xt[128:256, :])
            nc.sync.dma_start(bias_sb[:], biasd.ap())
            for c in range(2, HC):
                nc.sync.dma_start(xt_sb[:, c, :], xt[128 * c : 128 * (c + 1), :])
            wqt_r = wqt.ap().rearrange("(c p) m -> p c m", p=128)
            for c in range(HC):
                nc.sync.dma_start(wq_sb[:, c, :], wqt_r[:, c, :])
            nc.sync.dma_start(wo_sb[:], wot.ap().rearrange("(h p) n -> p h n", p=128))

            # ---------- phase 1: K/V/Q0 projections ----------
            # K uses 4 PSUM banks, V only 2 (two Sk-half passes) so head 0's
            # Q projection can accumulate in the remaining 2 banks DURING the
            # DMA-paced xt chunk stream: every chunk arrival feeds 10
            # matmuls, and the attention phase starts ~9us earlier.
            with tc.tile_pool(name="vtp", bufs=1) as vtp:
                vT_sb = vtp.tile([128, S], f16)
                with (
                    tc.tile_pool(name="kv_ps", bufs=1, space="PSUM") as kvps,
                    tc.tile_pool(name="q0_ps", bufs=1, space="PSUM") as q0ps,
                ):
                    kps = kvps.tile([128, 4, 512], f32, tag="kps")
                    vps = kvps.tile([128, 2, 512], f32, tag="vps")
                    q0 = q0ps.tile([128, SQ], f32, tag="q0")
                    for c in range(HC):
                        for sb in range(4):
                            nc.tensor.matmul(
                                kps[:, sb, :],
                                wk_sb[:, c, :],
                                xt_sb[:, c, 512 * sb : 512 * (sb + 1)],
                                start=(c == 0),
                                stop=(c == HC - 1),
                            )
                        for sb in range(2):
                            nc.tensor.matmul(
                                vps[:, sb, :],
                                wv_sb[:, c, :],
                                xt_sb[:, c, 512 * sb : 512 * (sb + 1)],
                                start=(c == 0),
                                stop=(c == HC - 1),
                            )
                        for sqh in range(2):
                            nc.tensor.matmul(
                                q0[:, 512 * sqh : 512 * (sqh + 1)],
                                wq_sb[:, c, 0:D],
                                xt_sb[:, c, 512 * sqh : 512 * (sqh + 1)],
                                start=(c == 0),
                                stop=(c == HC - 1),
                            )
                    # drain order: kT block 0 + qT(0) first (they gate the
                    # first scores), then vT half 0 (gates the transposes).
                    nc.vector.tensor_copy(kT_sb[:, 0:512], kps[:, 0, :])
                    nc.vector.tensor_copy(qT_sb[:, 0, :], q0[:])
                    for sb in range(2):
                        nc.vector.tensor_copy(
                            vT_sb[:, 512 * sb : 512 * (sb + 1)], vps[:, sb, :]
                        )
                    for sb in range(1, 4):
                        nc.vector.tensor_copy(
                            kT_sb[:, 512 * sb : 512 * (sb + 1)], kps[:, sb, :]
                        )
                    # V second half in the freed vps banks
                    vps2 = kvps.tile([128, 2, 512], f32, tag="vps")
                    for c in range(HC):
                        for sb in range(2):
                            nc.tensor.matmul(
                                vps2[:, sb, :],
                                wv_sb[:, c, :],
                                xt_sb[:, c, 512 * (sb + 2) : 512 * (sb + 3)],
                                start=(c == 0),
                                stop=(c == HC - 1),
                            )
                    for sb in range(2):
                        nc.vector.tensor_copy(
                            vT_sb[:, 512 * (sb + 2) : 512 * (sb + 3)],
                            vps2[:, sb, :],
                        )

                # ---------- phase 2: V transpose ----------
                with tc.tile_pool(name="proj_ps", bufs=2, space="PSUM") as pps:
                    for c in range(SKC):
                        pt = pps.tile([128, 128], f16, tag="vtr")
                        nc.tensor.transpose(
                            pt[:], vT_sb[:, 128 * c : 128 * (c + 1)], ident[:]
                        )
                        nc.vector.tensor_copy(v_sb[:, c, :], pt[:])

            # ---------- phase 3: attention (Q proj pipelined per head) ----
            with (
                tc.tile_pool(name="sc_ps", bufs=2, space="PSUM") as scp,
                tc.tile_pool(name="q_ps", bufs=1, space="PSUM") as qps,
                tc.tile_pool(name="av_ps", bufs=1, space="PSUM") as avp,
                tc.tile_pool(name="accp", bufs=2) as accp,
                tc.tile_pool(name="dnp", bufs=1) as dnp,
                tc.tile_pool(name="avsbp", bufs=2) as avsbp,
            ):
                def emit_scores(h, c):
                    # scores^T chunk [Sk 128, Sq 1024] = (K^T slice)^T Q^T,
                    # then e = exp(scale * scores + mask_bias) on the scalar
                    # engine (bias is per-Sk = per-partition).
                    sc = scp.tile([128, SQ], f32, tag="sc")
                    for sqh in range(2):
                        nc.tensor.matmul(
                            sc[:, 512 * sqh : 512 * (sqh + 1)],
                            kT_sb[:, 128 * c : 128 * (c + 1)],
                            qT_sb[:, h, 512 * sqh : 512 * (sqh + 1)],
                            start=True,
                            stop=True,
                        )
                    nc.scalar.activation(
                        e_sb[:, c, :],
                        sc[:],
                        Exp,
                        bias=bias_sb[:, c : c + 1],
                        scale=SCALE,
                    )

                def emit_qproj_mm(h):
                    # Q^T for head h (queries are columns 0..SQ-1 of the
                    # rolled xt); the PSUM->SBUF copy is emitted separately
                    # so its DVE slot lands after the previous head's
                    # normalization ops.
                    ps = qps.tile([128, SQ], f32, tag="qp")
                    for sqh in range(2):
                        for c in range(HC):
                            nc.tensor.matmul(
                                ps[:, 512 * sqh : 512 * (sqh + 1)],
                                wq_sb[:, c, D * h : D * (h + 1)],
                                xt_sb[:, c, 512 * sqh : 512 * (sqh + 1)],
                                start=(c == 0),
                                stop=(c == HC - 1),
                            )
                    return ps

                def emit_qproj_copy(h, ps):
                    nc.vector.tensor_copy(qT_sb[:, h, :], ps[:])

                # Normalization structure per head h (emitted in head h+1's
                # window so everything overlaps the matmul stream):
                #   rawcopy:   av PSUM -> SBUF (frees the single AV buffer)
                #   rs-mm:     ones-col matmul partition-sums the exp acc
                #              (emitted at the end of window h)
                #   rs copy:   [1,1024] PSUM -> SBUF fp16
                #   bc-mm:     ones-row matmul broadcasts it to 128 partitions
                #   recip+mul: DVE; recip is emitted before the qcopy so the
                #              sc-slot reuse by later score tiles never waits.
                def emit_rs(acc):
                    rs = scp.tile([128, SQ], f32, tag="sc")
                    for sqh in range(2):
                        nc.tensor.matmul(
                            rs[0:1, 512 * sqh : 512 * (sqh + 1)],
                            ones_col[:],
                            acc[:, 512 * sqh : 512 * (sqh + 1)],
                            start=True,
                            stop=True,
                        )
                    return rs

                def tail_pre(pav, pavsb, prs):
                    # window-start part: free AV PSUM, stage rowsums in SBUF
                    nc.vector.tensor_copy(pavsb[:], pav[:])
                    prs_sb = dnp.tile([1, SQ], f16, tag="rssb")
                    nc.vector.tensor_copy(prs_sb[:], prs[0:1, :])
                    return prs_sb

                def tail_bc(prs_sb):
                    bc = scp.tile([128, SQ], f32, tag="sc")
                    for sqh in range(2):
                        nc.tensor.matmul(
                            bc[:, 512 * sqh : 512 * (sqh + 1)],
                            ones_row[:],
                            prs_sb[:, 512 * sqh : 512 * (sqh + 1)],
                            start=True,
                            stop=True,
                        )
                    return bc

                def tail_norm(ph, pavsb, bc):
                    prden = dnp.tile([128, SQ], f32, tag="rden")
                    nc.vector.reciprocal_approx_fast(prden[:], bc[:])
                    nc.vector.tensor_mul(outT_sb[:, ph, :], pavsb[:], prden[:])

                def emit_qproj_part(h, ps, c0, c1):
                    for sqh in range(2):
                        for c in range(c0, c1):
                            nc.tensor.matmul(
                                ps[:, 512 * sqh : 512 * (sqh + 1)],
                                wq_sb[:, c, D * h : D * (h + 1)],
                                xt_sb[:, c, 512 * sqh : 512 * (sqh + 1)],
                                start=(c == 0),
                                stop=(c == HC - 1),
                            )

                def emit_av(av, c, stop):
                    for sqh in range(2):
                        nc.tensor.matmul(
                            av[:, 512 * sqh : 512 * (sqh + 1)],
                            v_sb[:, c, :],
                            e_sb[:, c, 512 * sqh : 512 * (sqh + 1)],
                            start=(c == 0),
                            stop=stop,
                        )

                def finish_head(pav, pacc):
                    # deferred last AV chunk (its exp is ready by now) and
                    # final acc fold, then the partition-sum matmuls
                    emit_av(pav, SKC - 1, True)
                    nc.vector.tensor_tensor(
                        pacc[0][:], e_sb[:, SKC - 1, :], pacc[1][:], Add
                    )

                prev = None
                for h in range(NH):
                    av = avp.tile([128, SQ], f32, tag="av")
                    accA = accp.tile([128, SQ], f16, tag="accA")
                    accB = accp.tile([128, SQ], f16, tag="accB")
                    acc = [accA, accB]
                    avsb = avsbp.tile([128, SQ], f16, tag="avsb")

                    emit_scores(h, 0)
                    qp_ps = (
                        qps.tile([128, SQ], f32, tag="qp", name=f"qp{h + 1}")
                        if h + 1 < NH
                        else None
                    )
                    if prev is not None:
                        # previous head's finish + normalization, interleaved
                        # with this head's Q-proj so the PE never waits on the
                        # scalar/DVE chain.
                        ph, pav, pacc, pavsb = prev
                        if qp_ps is not None:
                            emit_qproj_part(h + 1, qp_ps, 0, 2)
                        finish_head(pav, pacc)
                        if qp_ps is not None:
                            emit_qproj_part(h + 1, qp_ps, 2, 4)
                        prs = emit_rs(pacc[0])
                        nc.vector.tensor_copy(pavsb[:], pav[:])
                        prs_sb = dnp.tile([1, SQ], f16, tag="rssb")
                        nc.vector.tensor_copy(prs_sb[:], prs[0:1, :])
                        if qp_ps is not None:
                            emit_qproj_part(h + 1, qp_ps, 4, 6)
                        bc = tail_bc(prs_sb)
                        emit_scores(h, 1)
                        tail_norm(ph, pavsb, bc)
                        qp_done = 6
                    else:
                        if qp_ps is not None:
                            emit_qproj_part(h + 1, qp_ps, 0, 6)
                        emit_scores(h, 1)
                        qp_done = 6

                    # remaining Q-proj chunks are spread through the chunk
                    # loop: the exp stream (1.11us/chunk) is slower than the
                    # per-chunk sc+av matmuls (0.86us), so without filler the
                    # PE drains its runway and stalls ~100ns on every exp.
                    for c in range(SKC - 1):
                        emit_av(av, c, False)
                        if c + 2 < SKC:
                            emit_scores(h, c + 2)
                        if qp_ps is not None and c < 6:
                            emit_qproj_part(h + 1, qp_ps, qp_done + c, qp_done + c + 1)
                        if c == 6 and qp_ps is not None:
                            emit_qproj_copy(h + 1, qp_ps)
                        if c >= 1:
                            # acc[(c+1)%2] = e_c + (previous acc / e_0)
                            src1 = e_sb[:, 0, :] if c == 1 else acc[c % 2][:]
                            nc.vector.tensor_tensor(
                                acc[(c + 1) % 2][:], e_sb[:, c, :], src1, Add
                            )

                    prev = (h, av, acc, avsb)

                # epilogue: finish head 5 and normalize it
                lh, lav, lacc, lavsb = prev
                finish_head(lav, lacc)
                lrs = emit_rs(lacc[0])
                nc.vector.tensor_copy(lavsb[:], lav[:])
                lrs_sb = dnp.tile([1, SQ], f16, tag="rssb")
                nc.vector.tensor_copy(lrs_sb[:], lrs[0:1, :])
                tail_norm(lh, lavsb, tail_bc(lrs_sb))

            # ---------- phase 4: O-projection (partial; host sums groups) ----
            # The first tiles' h=0..4 contributions are emitted before any
            # h=5 matmul: head 5's outT arrives ~6us late (its normalization
            # chain runs after the last attention stream), and the in-order
            # PE would otherwise stall on each tile's 6th matmul.  All 8
            # PSUM banks are free here, so 6 tiles accumulate in parallel.
            with (
                tc.tile_pool(name="y_ps", bufs=6, space="PSUM") as yp,
                tc.tile_pool(name="y_sb", bufs=3) as ysb,
            ):
                tiles = [
                    (t, nb)
                    for t in range(SQ // 128)
                    for nb in range(HIDDEN // 512)
                ]
                NWARM = 6

                def omm(ps, t, nb, h0, h1):
                    for h in range(h0, h1):
                        nc.tensor.matmul(
                            ps[:],
                            outT_sb[:, h, 128 * t : 128 * (t + 1)],
                            wo_sb[:, h, 512 * nb : 512 * (nb + 1)],
                            start=(h == 0),
                            stop=(h == NH - 1),
                        )

                def ofinish(ps, t, nb):
                    ysb_t = ysb.tile([128, 512], f16, tag="ysb")
                    nc.vector.tensor_copy(ysb_t[:], ps[:])
                    nc.sync.dma_start(
                        y[128 * t : 128 * (t + 1), 512 * nb : 512 * (nb + 1)],
                        ysb_t[:],
                    )

                warm = []
                for t, nb in tiles[:NWARM]:
                    ps = yp.tile([128, 512], f32, tag="y", name=f"yw{t}_{nb}")
                    omm(ps, t, nb, 0, NH - 1)
                    warm.append((ps, t, nb))
                for ps, t, nb in warm:
                    omm(ps, t, nb, NH - 1, NH)
                    ofinish(ps, t, nb)
                for t, nb in tiles[NWARM:]:
                    ps = yp.tile([128, 512], f32, tag="y", name=f"y{t}_{nb}")
                    omm(ps, t, nb, 0, NH)
                    ofinish(ps, t, nb)

    nc.compile()
    return nc


def _get_nc():
    if "nc" not in _NC_CACHE:
        _NC_CACHE["nc"] = _build_nc()
    return _NC_CACHE["nc"]


def kernel(hidden_states, attention_mask, Wq, Wk, Wv, Wo):
    global last_results
    from concourse.bass_utils import run_bass_kernel_spmd

    f16 = np.float16
    hidden_states = np.asarray(hidden_states, dtype=np.float32)
    attention_mask = np.asarray(attention_mask, dtype=np.float32)
    Wq = np.asarray(Wq, dtype=np.float32)
    Wk = np.asarray(Wk, dtype=np.float32)
    Wv = np.asarray(Wv, dtype=np.float32)
    Wo = np.asarray(Wo, dtype=np.float32)

    nc = _get_nc()

    in_maps = []
    cores = []
    for b in range(2):
        xt_full = np.ascontiguousarray(hidden_states[b].T).astype(f16)  # [H, S]
        bias_full = ((1.0 - attention_mask[b]) * -10000.0).astype(np.float32)
        for hh in range(2):
            wqt = np.ascontiguousarray(
                Wq[NH * D * hh : NH * D * (hh + 1), :].T
            ).astype(f16)
            wkt = np.ascontiguousarray(Wk[D * hh : D * (hh + 1), :].T).astype(f16)
            wvt = np.ascontiguousarray(Wv[D * hh : D * (hh + 1), :].T).astype(f16)
            wot = np.ascontiguousarray(
                Wo[:, NH * D * hh : NH * D * (hh + 1)].T
            ).astype(f16)
            for th in range(2):
                # roll tokens so this core's queries are columns 0..SQ-1
                r = th * SQ
                xt_r = np.ascontiguousarray(
                    np.concatenate([xt_full[:, r:], xt_full[:, :r]], axis=1)
                )
                bias_r = np.concatenate([bias_full[r:], bias_full[:r]])
                biasd = np.ascontiguousarray(
                    bias_r.reshape(SKC, 128).T
                ).astype(np.float32)
                in_maps.append(
                    {
                        "xt": xt_r,
                        "wqt": wqt,
                        "wkt": wkt,
                        "wvt": wvt,
                        "wot": wot,
                        "biasd": biasd,
                    }
                )
                cores.append((b, hh, th))

    res = run_bass_kernel_spmd(nc, in_maps, core_ids=list(range(8)))
    last_results = res

    out = np.zeros((B, S, HIDDEN), dtype=np.float32)
    for (b, hh, th), r in zip(cores, res.results):
        out[b, th * SQ : (th + 1) * SQ, :] += r["y"].astype(np.float32)
    return out
